# revision 19
# baseline (speedup 1.0000x reference)
"""GNN edge-decoder kernel for Trainium2 (8 NeuronCores via Bass/Tile).

reference computes, for E=600000 edges over a [100000, 128] node table:
    node_emb = table[src_idx]              # gather   -> [E, 128]  (output)
    h  = relu(node_emb @ W1 + b1)          # [E, 64]
    h  = relu(h @ W2 + b2)                 # [E, 32]
    logits = (h @ W3 + b3).squeeze(-1)     # [E]      (output)
    returns (logits, labels, node_emb)     # labels passthrough

Sharding: edges are bucketed by src_idx // 12500 so core k only gathers
from rows [12500k, 12500(k+1)) of the table.  This keeps every per-core
gather index < 12500, inside the int16 range required by the hardware
dma_gather (InstDMAGatherAnt) instruction.  Each core receives its own
6.4MB slice of the table plus its (sorted, padded) local indices; the
host inverts the edge permutation on the way out.
"""

import numpy as np
from contextlib import ExitStack

import concourse.bacc as bacc
import concourse.bass as bass
import concourse.mybir as mybir
import concourse.tile as tile
from concourse.bass_utils import run_bass_kernel_spmd
from concourse.masks import make_identity

N_CORES = 8
N_NODES = 100000
D = 128
ROWS = N_NODES // N_CORES          # 12500 table rows per core
CHUNK = 4096                       # edges per gather
PAD_E = 19 * CHUNK                 # 77824 >= max edges/core (~75.6k)

F32 = mybir.dt.float32
I16 = mybir.dt.int16


def build_nc(rows=ROWS, pad_e=PAD_E, chunk=CHUNK):
    """Build the single-core Bass program (same NEFF runs SPMD on 8 cores)."""
    assert pad_e % chunk == 0 and chunk % 512 == 0
    n_chunks = pad_e // chunk
    jblk = chunk // 128                # 128-edge subtiles per chunk
    nseg = chunk // 512                # 512-edge matmul segments per chunk

    nc = bacc.Bacc("TRN2", debug=False, enable_asserts=False,
                   num_devices=N_CORES)

    tbl = nc.dram_tensor("tbl", [rows, D], F32, kind="ExternalInput").ap()
    idx = nc.dram_tensor("idx", [128, pad_e // 16], I16, kind="ExternalInput").ap()
    w1 = nc.dram_tensor("w1", [128, 64], F32, kind="ExternalInput").ap()
    b1 = nc.dram_tensor("b1", [128], F32, kind="ExternalInput").ap()   # b1 tiled x2
    w2 = nc.dram_tensor("w2", [128, 64], F32, kind="ExternalInput").ap()  # blockdiag(W2,W2)
    b2 = nc.dram_tensor("b2", [128], F32, kind="ExternalInput").ap()   # b2 tiled x4
    w3 = nc.dram_tensor("w3", [128, 4], F32, kind="ExternalInput").ap()   # blockdiag(W3 x4)
    b3 = nc.dram_tensor("b3", [4], F32, kind="ExternalInput").ap()     # b3 tiled x4
    emb = nc.dram_tensor("emb", [pad_e, D], F32, kind="ExternalOutput").ap()
    lgt = nc.dram_tensor("lgt", [pad_e], F32, kind="ExternalOutput").ap()

    # dma_gather lives in the "mlp" Q7 ucode library; Bacc auto-inserts the
    # library reload when it sees InstDMAGatherAnt.
    with ExitStack() as ctx:
        tc = ctx.enter_context(tile.TileContext(nc))
        consts = ctx.enter_context(tc.tile_pool(name="consts", bufs=1))
        gp = ctx.enter_context(tc.tile_pool(name="g", bufs=3))
        xp = ctx.enter_context(tc.tile_pool(name="xT", bufs=2))
        hp = ctx.enter_context(tc.tile_pool(name="h", bufs=3))
        lp = ctx.enter_context(tc.tile_pool(name="lg", bufs=2))
        pst = ctx.enter_context(tc.tile_pool(name="pst", bufs=2, space="PSUM"))
        ps1 = ctx.enter_context(tc.tile_pool(name="ps1", bufs=2, space="PSUM"))
        ps2 = ctx.enter_context(tc.tile_pool(name="ps2", bufs=2, space="PSUM"))
        ps3 = ctx.enter_context(tc.tile_pool(name="ps3", bufs=2, space="PSUM"))

        ident = consts.tile([128, 128], F32)
        make_identity(nc, ident[:])
        idx_t = consts.tile([128, pad_e // 16], I16)
        nc.sync.dma_start(idx_t[:], idx[:, :])
        w1_t = consts.tile([128, 64], F32)
        nc.sync.dma_start(w1_t[:], w1[:, :])
        w2_t = consts.tile([128, 64], F32)
        nc.sync.dma_start(w2_t[:], w2[:, :])
        w3_t = consts.tile([128, 4], F32)
        nc.sync.dma_start(w3_t[:], w3[:, :])
        b1_t = consts.tile([128, 1], F32)
        nc.sync.dma_start(b1_t[:], b1[:, None])
        b2_t = consts.tile([128, 1], F32)
        nc.sync.dma_start(b2_t[:], b2[:, None])
        b3_t = consts.tile([4, 1], F32)
        nc.sync.dma_start(b3_t[:], b3[:, None])

        for c in range(n_chunks):
            # --- gather: edge i of chunk -> partition i%128, block i//128
            g = gp.tile([128, chunk], F32)
            g3 = g[:].rearrange("p (j f) -> p j f", f=D)
            nc.gpsimd.dma_gather(
                g3, tbl[:, :], idx_t[:, c * (chunk // 16):(c + 1) * (chunk // 16)],
                chunk, chunk, D, single_packet=False,
            )
            # --- node_emb writeback (512B descriptors, contiguous rows)
            nc.sync.dma_start(
                emb[c * chunk:(c + 1) * chunk, :].rearrange("(j p) f -> p j f", p=128),
                g3,
            )
            # --- transpose to feature-major xT[128f, chunk e]
            xT = xp.tile([128, chunk], F32)
            for jq in range(jblk // 4):
                pt = pst.tile([128, 512], F32)
                for u in range(4):
                    j = jq * 4 + u
                    nc.tensor.transpose(
                        pt[:, u * 128:(u + 1) * 128],
                        g[:, j * 128:(j + 1) * 128],
                        ident[:],
                    )
                nc.vector.tensor_copy(xT[:, jq * 512:(jq + 1) * 512], pt[:])
            # --- MLP: 512-edge segments; h1 pairs stacked [128,512];
            #     mm2 = blockdiag(W2,W2) maps a pair in one matmul;
            #     h2 quads stacked [128,512]; mm3 = blockdiag(W3 x4) -> [4,512]
            for half in range(nseg // 4):
                h2p = ps2.tile([128, 512], F32)
                for sp in range(2):
                    h1p = ps1.tile([128, 512], F32)
                    for u in range(2):
                        s = half * 4 + sp * 2 + u
                        nc.tensor.matmul(
                            h1p[u * 64:(u + 1) * 64, :], w1_t[:],
                            xT[:, s * 512:(s + 1) * 512], start=True, stop=True,
                        )
                    h1s = hp.tile([128, 512], F32)
                    nc.scalar.activation(h1s[:], h1p[:],
                                         mybir.ActivationFunctionType.Relu,
                                         bias=b1_t[:])
                    nc.tensor.matmul(
                        h2p[sp * 64:(sp + 1) * 64, :], w2_t[:], h1s[:],
                        start=True, stop=True,
                    )
                h2s = hp.tile([128, 512], F32)
                nc.scalar.activation(h2s[:], h2p[:],
                                     mybir.ActivationFunctionType.Relu,
                                     bias=b2_t[:])
                lgp = ps3.tile([4, 512], F32)
                nc.tensor.matmul(lgp[:], w3_t[:], h2s[:], start=True, stop=True)
                lgs = lp.tile([4, 512], F32)
                nc.scalar.add(lgs[:], lgp[:], b3_t[:])
                nc.sync.dma_start(
                    lgt[c * chunk + half * 2048:
                        c * chunk + (half + 1) * 2048].rearrange("(s i) -> s i", i=512),
                    lgs[:],
                )
    nc.compile()
    return nc


_NC_CACHE = {}


def _get_nc():
    if "nc" not in _NC_CACHE:
        _NC_CACHE["nc"] = build_nc()
    return _NC_CACHE["nc"]


def shard_inputs(block_outputs, src_idx, W1, b1, W2, b2, W3, b3):
    """Host-side sharding. Returns (in_maps, order, counts)."""
    src = np.asarray(src_idx).astype(np.int64)
    e_total = src.shape[0]
    order = np.argsort(src, kind="stable")   # groups by shard AND sorts for locality
    shard = src // ROWS                      # bucket per edge
    counts = np.bincount(shard, minlength=N_CORES)
    assert counts.max() <= PAD_E, f"shard overflow: {counts.max()} > {PAD_E}"
    offs = np.zeros(N_CORES + 1, np.int64)
    np.cumsum(counts, out=offs[1:])
    local_sorted = (src[order] % ROWS).astype(np.int16)

    bo = np.ascontiguousarray(np.asarray(block_outputs, dtype=np.float32))
    W1 = np.ascontiguousarray(np.asarray(W1, np.float32))
    W2 = np.asarray(W2, np.float32)
    W3 = np.asarray(W3, np.float32)
    w2blk = np.zeros((128, 64), np.float32)          # blockdiag(W2, W2)
    w2blk[0:64, 0:32] = W2
    w2blk[64:128, 32:64] = W2
    w3blk = np.zeros((128, 4), np.float32)           # blockdiag(W3 x 4)
    for q in range(4):
        w3blk[32 * q:32 * (q + 1), q] = W3[:, 0]
    b1r = np.ascontiguousarray(np.tile(np.asarray(b1, np.float32), 2))
    b2r = np.ascontiguousarray(np.tile(np.asarray(b2, np.float32), 4))
    b3r = np.ascontiguousarray(np.tile(np.asarray(b3, np.float32), 4))

    in_maps = []
    for k in range(N_CORES):
        lk = local_sorted[offs[k]:offs[k + 1]]
        lpad = np.zeros(PAD_E, np.int16)
        lpad[:lk.shape[0]] = lk
        wrapped = lpad.reshape(PAD_E // 16, 16).T          # [16, PAD_E/16]
        idx128 = np.ascontiguousarray(np.tile(wrapped, (8, 1)))  # replicate to 128p
        in_maps.append({
            "tbl": np.ascontiguousarray(bo[k * ROWS:(k + 1) * ROWS]),
            "idx": idx128,
            "w1": W1, "b1": b1r, "w2": w2blk, "b2": b2r, "w3": w3blk, "b3": b3r,
        })
    return in_maps, order, counts, offs, e_total


def kernel(block_outputs, src_idx, labels, W1, b1, W2, b2, W3, b3,
           _trace=False, _trace_kwargs=None):
    in_maps, order, counts, offs, e_total = shard_inputs(
        block_outputs, src_idx, W1, b1, W2, b2, W3, b3)
    nc = _get_nc()
    res = run_bass_kernel_spmd(nc, in_maps, list(range(N_CORES)),
                               trace=_trace, **(_trace_kwargs or {}))
    emb_sorted = np.concatenate(
        [res.results[k]["emb"][:counts[k]] for k in range(N_CORES)], axis=0)
    lgt_sorted = np.concatenate(
        [res.results[k]["lgt"][:counts[k]] for k in range(N_CORES)], axis=0)
    node_emb = np.empty((e_total, D), np.float32)
    node_emb[order] = emb_sorted
    logits = np.empty((e_total,), np.float32)
    logits[order] = lgt_sorted
    labels = np.asarray(labels, np.float32)
    if _trace:
        kernel._last_results = res
    return logits, labels, node_emb


# revision 21
# speedup vs baseline: 1.1100x; 1.1100x over previous
"""GNN edge-decoder kernel for Trainium2 (8 NeuronCores via Bass/Tile).

reference computes, for E=600000 edges over a [100000, 128] node table:
    node_emb = table[src_idx]              # gather   -> [E, 128]  (output)
    h  = relu(node_emb @ W1 + b1)          # [E, 64]
    h  = relu(h @ W2 + b2)                 # [E, 32]
    logits = (h @ W3 + b3).squeeze(-1)     # [E]      (output)
    returns (logits, labels, node_emb)     # labels passthrough

Sharding: edges are bucketed by src_idx // 12500 so core k only gathers
from rows [12500k, 12500(k+1)) of the table.  This keeps every per-core
gather index < 12500, inside the int16 range required by the hardware
dma_gather (InstDMAGatherAnt) instruction.  Each core receives its own
6.4MB slice of the table plus its (sorted, padded) local indices; the
host inverts the edge permutation on the way out.
"""

import numpy as np
from contextlib import ExitStack

import concourse.bacc as bacc
import concourse.bass as bass
import concourse.mybir as mybir
import concourse.tile as tile
from concourse.bass_utils import run_bass_kernel_spmd
from concourse.masks import make_identity

N_CORES = 8
N_NODES = 100000
D = 128
ROWS = N_NODES // N_CORES          # 12500 table rows per core
CHUNK = 4096                       # edges per gather
PAD_E = 19 * CHUNK                 # 77824 >= max edges/core (~75.6k)

F32 = mybir.dt.float32
I16 = mybir.dt.int16


def build_nc(rows=ROWS, pad_e=PAD_E, chunk=CHUNK, queues=4):
    """Build the single-core Bass program (same NEFF runs SPMD on 8 cores)."""
    assert pad_e % chunk == 0 and chunk % 512 == 0
    n_chunks = pad_e // chunk
    jblk = chunk // 128                # 128-edge subtiles per chunk
    nseg = chunk // 512                # 512-edge matmul segments per chunk

    nc = bacc.Bacc("TRN2", debug=False, enable_asserts=False,
                   num_devices=N_CORES, num_swdge_queues=queues)

    tbl = nc.dram_tensor("tbl", [rows, D], F32, kind="ExternalInput").ap()
    idx = nc.dram_tensor("idx", [128, pad_e // 16], I16, kind="ExternalInput").ap()
    w1 = nc.dram_tensor("w1", [128, 64], F32, kind="ExternalInput").ap()
    b1 = nc.dram_tensor("b1", [128], F32, kind="ExternalInput").ap()   # b1 tiled x2
    w2 = nc.dram_tensor("w2", [128, 64], F32, kind="ExternalInput").ap()  # blockdiag(W2,W2)
    b2 = nc.dram_tensor("b2", [128], F32, kind="ExternalInput").ap()   # b2 tiled x4
    w3 = nc.dram_tensor("w3", [128, 4], F32, kind="ExternalInput").ap()   # blockdiag(W3 x4)
    b3 = nc.dram_tensor("b3", [4], F32, kind="ExternalInput").ap()     # b3 tiled x4
    emb = nc.dram_tensor("emb", [pad_e, D], F32, kind="ExternalOutput").ap()
    lgt = nc.dram_tensor("lgt", [pad_e], F32, kind="ExternalOutput").ap()

    # dma_gather lives in the "mlp" Q7 ucode library; Bacc auto-inserts the
    # library reload when it sees InstDMAGatherAnt.
    with ExitStack() as ctx:
        tc = ctx.enter_context(tile.TileContext(nc))
        consts = ctx.enter_context(tc.tile_pool(name="consts", bufs=1))
        gp = ctx.enter_context(tc.tile_pool(name="g", bufs=3))
        xp = ctx.enter_context(tc.tile_pool(name="xT", bufs=2))
        hp = ctx.enter_context(tc.tile_pool(name="h", bufs=3))
        lp = ctx.enter_context(tc.tile_pool(name="lg", bufs=2))
        pst = ctx.enter_context(tc.tile_pool(name="pst", bufs=2, space="PSUM"))
        ps1 = ctx.enter_context(tc.tile_pool(name="ps1", bufs=2, space="PSUM"))
        ps2 = ctx.enter_context(tc.tile_pool(name="ps2", bufs=2, space="PSUM"))
        ps3 = ctx.enter_context(tc.tile_pool(name="ps3", bufs=2, space="PSUM"))

        ident = consts.tile([128, 128], F32)
        make_identity(nc, ident[:])
        idx_t = consts.tile([128, pad_e // 16], I16)
        nc.sync.dma_start(idx_t[:], idx[:, :])
        w1_t = consts.tile([128, 64], F32)
        nc.sync.dma_start(w1_t[:], w1[:, :])
        w2_t = consts.tile([128, 64], F32)
        nc.sync.dma_start(w2_t[:], w2[:, :])
        w3_t = consts.tile([128, 4], F32)
        nc.sync.dma_start(w3_t[:], w3[:, :])
        b1_t = consts.tile([128, 1], F32)
        nc.sync.dma_start(b1_t[:], b1[:, None])
        b2_t = consts.tile([128, 1], F32)
        nc.sync.dma_start(b2_t[:], b2[:, None])
        b3_t = consts.tile([4, 1], F32)
        nc.sync.dma_start(b3_t[:], b3[:, None])

        for c in range(n_chunks):
            # --- gather: edge i of chunk -> partition i%128, block i//128
            g = gp.tile([128, chunk], F32)
            g3 = g[:].rearrange("p (j f) -> p j f", f=D)
            nc.gpsimd.dma_gather(
                g3, tbl[:, :], idx_t[:, c * (chunk // 16):(c + 1) * (chunk // 16)],
                chunk, chunk, D, single_packet=False, queue_num=c % queues,
            )
            # --- node_emb writeback (512B descriptors, contiguous rows)
            nc.sync.dma_start(
                emb[c * chunk:(c + 1) * chunk, :].rearrange("(j p) f -> p j f", p=128),
                g3,
            )
            # --- transpose to feature-major xT[128f, chunk e]
            xT = xp.tile([128, chunk], F32)
            for jq in range(jblk // 4):
                pt = pst.tile([128, 512], F32)
                for u in range(4):
                    j = jq * 4 + u
                    nc.tensor.transpose(
                        pt[:, u * 128:(u + 1) * 128],
                        g[:, j * 128:(j + 1) * 128],
                        ident[:],
                    )
                nc.vector.tensor_copy(xT[:, jq * 512:(jq + 1) * 512], pt[:])
            # --- MLP: 512-edge segments; h1 pairs stacked [128,512];
            #     mm2 = blockdiag(W2,W2) maps a pair in one matmul;
            #     h2 quads stacked [128,512]; mm3 = blockdiag(W3 x4) -> [4,512]
            for half in range(nseg // 4):
                h2p = ps2.tile([128, 512], F32)
                for sp in range(2):
                    h1p = ps1.tile([128, 512], F32)
                    for u in range(2):
                        s = half * 4 + sp * 2 + u
                        nc.tensor.matmul(
                            h1p[u * 64:(u + 1) * 64, :], w1_t[:],
                            xT[:, s * 512:(s + 1) * 512], start=True, stop=True,
                        )
                    h1s = hp.tile([128, 512], F32)
                    nc.scalar.activation(h1s[:], h1p[:],
                                         mybir.ActivationFunctionType.Relu,
                                         bias=b1_t[:])
                    nc.tensor.matmul(
                        h2p[sp * 64:(sp + 1) * 64, :], w2_t[:], h1s[:],
                        start=True, stop=True,
                    )
                h2s = hp.tile([128, 512], F32)
                nc.scalar.activation(h2s[:], h2p[:],
                                     mybir.ActivationFunctionType.Relu,
                                     bias=b2_t[:])
                lgp = ps3.tile([4, 512], F32)
                nc.tensor.matmul(lgp[:], w3_t[:], h2s[:], start=True, stop=True)
                lgs = lp.tile([4, 512], F32)
                nc.scalar.add(lgs[:], lgp[:], b3_t[:])
                nc.sync.dma_start(
                    lgt[c * chunk + half * 2048:
                        c * chunk + (half + 1) * 2048].rearrange("(s i) -> s i", i=512),
                    lgs[:],
                )
    nc.compile()
    return nc


_NC_CACHE = {}


def _get_nc():
    if "nc" not in _NC_CACHE:
        _NC_CACHE["nc"] = build_nc()
    return _NC_CACHE["nc"]


def shard_inputs(block_outputs, src_idx, W1, b1, W2, b2, W3, b3):
    """Host-side sharding. Returns (in_maps, order, counts)."""
    src = np.asarray(src_idx).astype(np.int64)
    e_total = src.shape[0]
    order = np.argsort(src, kind="stable")   # groups by shard AND sorts for locality
    shard = src // ROWS                      # bucket per edge
    counts = np.bincount(shard, minlength=N_CORES)
    assert counts.max() <= PAD_E, f"shard overflow: {counts.max()} > {PAD_E}"
    offs = np.zeros(N_CORES + 1, np.int64)
    np.cumsum(counts, out=offs[1:])
    local_sorted = (src[order] % ROWS).astype(np.int16)

    bo = np.ascontiguousarray(np.asarray(block_outputs, dtype=np.float32))
    W1 = np.ascontiguousarray(np.asarray(W1, np.float32))
    W2 = np.asarray(W2, np.float32)
    W3 = np.asarray(W3, np.float32)
    w2blk = np.zeros((128, 64), np.float32)          # blockdiag(W2, W2)
    w2blk[0:64, 0:32] = W2
    w2blk[64:128, 32:64] = W2
    w3blk = np.zeros((128, 4), np.float32)           # blockdiag(W3 x 4)
    for q in range(4):
        w3blk[32 * q:32 * (q + 1), q] = W3[:, 0]
    b1r = np.ascontiguousarray(np.tile(np.asarray(b1, np.float32), 2))
    b2r = np.ascontiguousarray(np.tile(np.asarray(b2, np.float32), 4))
    b3r = np.ascontiguousarray(np.tile(np.asarray(b3, np.float32), 4))

    in_maps = []
    for k in range(N_CORES):
        lk = local_sorted[offs[k]:offs[k + 1]]
        lpad = np.zeros(PAD_E, np.int16)
        lpad[:lk.shape[0]] = lk
        wrapped = lpad.reshape(PAD_E // 16, 16).T          # [16, PAD_E/16]
        idx128 = np.ascontiguousarray(np.tile(wrapped, (8, 1)))  # replicate to 128p
        in_maps.append({
            "tbl": np.ascontiguousarray(bo[k * ROWS:(k + 1) * ROWS]),
            "idx": idx128,
            "w1": W1, "b1": b1r, "w2": w2blk, "b2": b2r, "w3": w3blk, "b3": b3r,
        })
    return in_maps, order, counts, offs, e_total


def kernel(block_outputs, src_idx, labels, W1, b1, W2, b2, W3, b3,
           _trace=False, _trace_kwargs=None):
    in_maps, order, counts, offs, e_total = shard_inputs(
        block_outputs, src_idx, W1, b1, W2, b2, W3, b3)
    nc = _get_nc()
    res = run_bass_kernel_spmd(nc, in_maps, list(range(N_CORES)),
                               trace=_trace, **(_trace_kwargs or {}))
    emb_sorted = np.concatenate(
        [res.results[k]["emb"][:counts[k]] for k in range(N_CORES)], axis=0)
    lgt_sorted = np.concatenate(
        [res.results[k]["lgt"][:counts[k]] for k in range(N_CORES)], axis=0)
    node_emb = np.empty((e_total, D), np.float32)
    node_emb[order] = emb_sorted
    logits = np.empty((e_total,), np.float32)
    logits[order] = lgt_sorted
    labels = np.asarray(labels, np.float32)
    if _trace:
        kernel._last_results = res
    return logits, labels, node_emb


# revision 25
# speedup vs baseline: 1.2044x; 1.0851x over previous
"""GNN edge-decoder kernel for Trainium2 (8 NeuronCores via Bass/Tile).

reference computes, for E=600000 edges over a [100000, 128] node table:
    node_emb = table[src_idx]              # gather   -> [E, 128]  (output)
    h  = relu(node_emb @ W1 + b1)          # [E, 64]
    h  = relu(h @ W2 + b2)                 # [E, 32]
    logits = (h @ W3 + b3).squeeze(-1)     # [E]      (output)
    returns (logits, labels, node_emb)     # labels passthrough

Sharding: edges are bucketed by src_idx // 12500 so core k only gathers
from rows [12500k, 12500(k+1)) of the table.  This keeps every per-core
gather index < 12500, inside the int16 range required by the hardware
dma_gather (InstDMAGatherAnt) instruction.  Each core receives its own
6.4MB slice of the table plus its (sorted, padded) local indices; the
host inverts the edge permutation on the way out.
"""

import numpy as np
from contextlib import ExitStack

import concourse.bacc as bacc
import concourse.bass as bass
import concourse.mybir as mybir
import concourse.tile as tile
from concourse.bass_utils import run_bass_kernel_spmd
from concourse.masks import make_identity

N_CORES = 8
N_NODES = 100000
D = 128
ROWS = N_NODES // N_CORES          # 12500 table rows per core
CHUNK = 4096                       # edges per gather
PAD_E = 19 * CHUNK                 # 77824 >= max edges/core (~75.6k)

F32 = mybir.dt.float32
I16 = mybir.dt.int16


def build_nc(rows=ROWS, pad_e=PAD_E, chunk=CHUNK, queues=4,
             gbufs=3, xbufs=2, qpat=None):
    """Build the single-core Bass program (same NEFF runs SPMD on 8 cores)."""
    assert pad_e % chunk == 0 and chunk % 512 == 0
    n_chunks = pad_e // chunk
    jblk = chunk // 128                # 128-edge subtiles per chunk
    nseg = chunk // 512                # 512-edge matmul segments per chunk
    if qpat is None:
        qpat = [c % queues for c in range(n_chunks)]

    nc = bacc.Bacc("TRN2", debug=False, enable_asserts=False,
                   num_devices=N_CORES, num_swdge_queues=queues)

    tbl = nc.dram_tensor("tbl", [rows, D], F32, kind="ExternalInput").ap()
    idx = nc.dram_tensor("idx", [128, pad_e // 16], I16, kind="ExternalInput").ap()
    w1 = nc.dram_tensor("w1", [128, 64], F32, kind="ExternalInput").ap()
    b1 = nc.dram_tensor("b1", [128], F32, kind="ExternalInput").ap()   # b1 tiled x2
    w2 = nc.dram_tensor("w2", [128, 64], F32, kind="ExternalInput").ap()  # blockdiag(W2,W2)
    b2 = nc.dram_tensor("b2", [128], F32, kind="ExternalInput").ap()   # b2 tiled x4
    w3 = nc.dram_tensor("w3", [128, 4], F32, kind="ExternalInput").ap()   # blockdiag(W3 x4)
    b3 = nc.dram_tensor("b3", [4], F32, kind="ExternalInput").ap()     # b3 tiled x4
    emb = nc.dram_tensor("emb", [pad_e, D], F32, kind="ExternalOutput").ap()
    lgt = nc.dram_tensor("lgt", [pad_e], F32, kind="ExternalOutput").ap()

    # dma_gather lives in the "mlp" Q7 ucode library; Bacc auto-inserts the
    # library reload when it sees InstDMAGatherAnt.
    with ExitStack() as ctx:
        tc = ctx.enter_context(tile.TileContext(nc))
        consts = ctx.enter_context(tc.tile_pool(name="consts", bufs=1))
        gp = ctx.enter_context(tc.tile_pool(name="g", bufs=gbufs))
        xp = ctx.enter_context(tc.tile_pool(name="xT", bufs=xbufs))
        hp = ctx.enter_context(tc.tile_pool(name="h", bufs=3))
        lp = ctx.enter_context(tc.tile_pool(name="lg", bufs=2))
        pst = ctx.enter_context(tc.tile_pool(name="pst", bufs=2, space="PSUM"))
        ps1 = ctx.enter_context(tc.tile_pool(name="ps1", bufs=2, space="PSUM"))
        ps2 = ctx.enter_context(tc.tile_pool(name="ps2", bufs=2, space="PSUM"))
        ps3 = ctx.enter_context(tc.tile_pool(name="ps3", bufs=2, space="PSUM"))

        ident = consts.tile([128, 128], F32)
        make_identity(nc, ident[:])
        idx_t = consts.tile([128, pad_e // 16], I16)
        nc.sync.dma_start(idx_t[:], idx[:, :])
        w1_t = consts.tile([128, 64], F32)
        nc.sync.dma_start(w1_t[:], w1[:, :])
        w2_t = consts.tile([128, 64], F32)
        nc.sync.dma_start(w2_t[:], w2[:, :])
        w3_t = consts.tile([128, 4], F32)
        nc.sync.dma_start(w3_t[:], w3[:, :])
        b1_t = consts.tile([128, 1], F32)
        nc.sync.dma_start(b1_t[:], b1[:, None])
        b2_t = consts.tile([128, 1], F32)
        nc.sync.dma_start(b2_t[:], b2[:, None])
        b3_t = consts.tile([4, 1], F32)
        nc.sync.dma_start(b3_t[:], b3[:, None])

        for c in range(n_chunks):
            # --- gather: edge i of chunk -> partition i%128, block i//128
            g = gp.tile([128, chunk], F32)
            g3 = g[:].rearrange("p (j f) -> p j f", f=D)
            nc.gpsimd.dma_gather(
                g3, tbl[:, :], idx_t[:, c * (chunk // 16):(c + 1) * (chunk // 16)],
                chunk, chunk, D, single_packet=False, queue_num=qpat[c],
            )
            # --- node_emb writeback (512B descriptors, contiguous rows)
            nc.sync.dma_start(
                emb[c * chunk:(c + 1) * chunk, :].rearrange("(j p) f -> p j f", p=128),
                g3,
            )
            # --- transpose to feature-major xT[128f, chunk e]
            xT = xp.tile([128, chunk], F32)
            for jq in range(jblk // 4):
                pt = pst.tile([128, 512], F32)
                for u in range(4):
                    j = jq * 4 + u
                    nc.tensor.transpose(
                        pt[:, u * 128:(u + 1) * 128],
                        g[:, j * 128:(j + 1) * 128],
                        ident[:],
                    )
                nc.vector.tensor_copy(xT[:, jq * 512:(jq + 1) * 512], pt[:])
            # --- MLP: 512-edge segments; h1 pairs stacked [128,512];
            #     mm2 = blockdiag(W2,W2) maps a pair in one matmul;
            #     h2 quads stacked [128,512]; mm3 = blockdiag(W3 x4) -> [4,512]
            for half in range(nseg // 4):
                h2p = ps2.tile([128, 512], F32)
                for sp in range(2):
                    h1p = ps1.tile([128, 512], F32)
                    for u in range(2):
                        s = half * 4 + sp * 2 + u
                        nc.tensor.matmul(
                            h1p[u * 64:(u + 1) * 64, :], w1_t[:],
                            xT[:, s * 512:(s + 1) * 512], start=True, stop=True,
                        )
                    h1s = hp.tile([128, 512], F32)
                    nc.scalar.activation(h1s[:], h1p[:],
                                         mybir.ActivationFunctionType.Relu,
                                         bias=b1_t[:])
                    nc.tensor.matmul(
                        h2p[sp * 64:(sp + 1) * 64, :], w2_t[:], h1s[:],
                        start=True, stop=True,
                    )
                h2s = hp.tile([128, 512], F32)
                nc.scalar.activation(h2s[:], h2p[:],
                                     mybir.ActivationFunctionType.Relu,
                                     bias=b2_t[:])
                lgp = ps3.tile([4, 512], F32)
                nc.tensor.matmul(lgp[:], w3_t[:], h2s[:], start=True, stop=True)
                lgs = lp.tile([4, 512], F32)
                nc.scalar.add(lgs[:], lgp[:], b3_t[:])
                nc.sync.dma_start(
                    lgt[c * chunk + half * 2048:
                        c * chunk + (half + 1) * 2048].rearrange("(s i) -> s i", i=512),
                    lgs[:],
                )
    nc.compile()
    return nc


_NC_CACHE = {}


def _get_nc():
    if "nc" not in _NC_CACHE:
        import os
        kw = {}
        if os.environ.get("KGBUFS"):
            kw["gbufs"] = int(os.environ["KGBUFS"])
        if os.environ.get("KXBUFS"):
            kw["xbufs"] = int(os.environ["KXBUFS"])
        if os.environ.get("KQPAT"):
            kw["qpat"] = [int(x) for x in os.environ["KQPAT"].split(",")]
            kw["qpat"] = (kw["qpat"] * PAD_E)[:PAD_E // CHUNK]
        if os.environ.get("KCHUNK"):
            kw["chunk"] = int(os.environ["KCHUNK"])
        _NC_CACHE["nc"] = build_nc(**kw)
    return _NC_CACHE["nc"]


def shard_inputs(block_outputs, src_idx, W1, b1, W2, b2, W3, b3):
    """Host-side sharding. Returns (in_maps, order, counts)."""
    src = np.asarray(src_idx).astype(np.int64)
    e_total = src.shape[0]
    order = np.argsort(src, kind="stable")   # groups by shard AND sorts for locality
    shard = src // ROWS                      # bucket per edge
    counts = np.bincount(shard, minlength=N_CORES)
    assert counts.max() <= PAD_E, f"shard overflow: {counts.max()} > {PAD_E}"
    offs = np.zeros(N_CORES + 1, np.int64)
    np.cumsum(counts, out=offs[1:])
    local_sorted = (src[order] % ROWS).astype(np.int16)

    bo = np.ascontiguousarray(np.asarray(block_outputs, dtype=np.float32))
    W1 = np.ascontiguousarray(np.asarray(W1, np.float32))
    W2 = np.asarray(W2, np.float32)
    W3 = np.asarray(W3, np.float32)
    w2blk = np.zeros((128, 64), np.float32)          # blockdiag(W2, W2)
    w2blk[0:64, 0:32] = W2
    w2blk[64:128, 32:64] = W2
    w3blk = np.zeros((128, 4), np.float32)           # blockdiag(W3 x 4)
    for q in range(4):
        w3blk[32 * q:32 * (q + 1), q] = W3[:, 0]
    b1r = np.ascontiguousarray(np.tile(np.asarray(b1, np.float32), 2))
    b2r = np.ascontiguousarray(np.tile(np.asarray(b2, np.float32), 4))
    b3r = np.ascontiguousarray(np.tile(np.asarray(b3, np.float32), 4))

    in_maps = []
    for k in range(N_CORES):
        lk = local_sorted[offs[k]:offs[k + 1]]
        lpad = np.zeros(PAD_E, np.int16)
        lpad[:lk.shape[0]] = lk
        wrapped = lpad.reshape(PAD_E // 16, 16).T          # [16, PAD_E/16]
        idx128 = np.ascontiguousarray(np.tile(wrapped, (8, 1)))  # replicate to 128p
        in_maps.append({
            "tbl": np.ascontiguousarray(bo[k * ROWS:(k + 1) * ROWS]),
            "idx": idx128,
            "w1": W1, "b1": b1r, "w2": w2blk, "b2": b2r, "w3": w3blk, "b3": b3r,
        })
    return in_maps, order, counts, offs, e_total


def kernel(block_outputs, src_idx, labels, W1, b1, W2, b2, W3, b3,
           _trace=False, _trace_kwargs=None):
    in_maps, order, counts, offs, e_total = shard_inputs(
        block_outputs, src_idx, W1, b1, W2, b2, W3, b3)
    nc = _get_nc()
    res = run_bass_kernel_spmd(nc, in_maps, list(range(N_CORES)),
                               trace=_trace, **(_trace_kwargs or {}))
    emb_sorted = np.concatenate(
        [res.results[k]["emb"][:counts[k]] for k in range(N_CORES)], axis=0)
    lgt_sorted = np.concatenate(
        [res.results[k]["lgt"][:counts[k]] for k in range(N_CORES)], axis=0)
    node_emb = np.empty((e_total, D), np.float32)
    node_emb[order] = emb_sorted
    logits = np.empty((e_total,), np.float32)
    logits[order] = lgt_sorted
    labels = np.asarray(labels, np.float32)
    if _trace:
        kernel._last_results = res
    return logits, labels, node_emb
